# revision 8
# baseline (speedup 1.0000x reference)
"""VQ codebook soft-assignment encoding kernel for 8 trn2 NeuronCores.

Math (per batch b):
  Xf = X[b].reshape(D, N).T                        # [N, D], N = H*W
  logit[n,k] = scale[k] * (||x_n||^2 - 2<x_n,c_k> + ||c_k||^2)
  A = softmax(logit, axis=k)
  E[b,k,:] = sum_n A[n,k] * (x_n - c_k)            # [K, D]

Sharding: data-parallel over B (4 batches per core), codewords/scale replicated.

Device plan per core (heavy compute fp16, fp32 PSUM accumulation):
  - SWDGE cast-load X[b] fp32 HBM -> fp16 SBUF, one DMA per batch, [p, h, n]
  - logits in [k, n] layout:  PSUM = G^T X + S^T X^2, U = exp(PSUM + bias_k)
  - transposes via SBUF-source dma_gather(transpose=True) on SWDGE:
    these are InstDMAGatherAnt, which the tile scheduler does NOT serialize
    against other DMA traffic (unlike InstDmaTransposeAnt, which gets a
    global DMA-fence before and after).
      X  -> XT  [128, 72, 256]: token d reads x16[d%128, d//128, :]
      U  -> UTG [128, 18, 128]: token q*32+k reads u16[k, q*2304 : +2304]
  - normalize on DVE: den = reduce_k, A = U * (1/den)
  - E-matmul on PE: psE[32, 257] += A_t^T @ XT_t over 72 n-chunks
  - E = psE[:, :256] - S_k * c  (DVE), DMA out fp32
"""
import numpy as np
from contextlib import ExitStack

import concourse.bass as bass
import concourse.mybir as mybir
import concourse.tile as tile
from concourse.tile import ScopedClock
from concourse.bass_utils import run_bass_kernel_spmd

dt = mybir.dt

B, D, K, H, W = 32, 256, 32, 96, 96
N = H * W                 # 9216
NCORES = 8
BPC = B // NCORES         # 4 batches per core
TN = 512                  # n-tile for logits pass
NT = N // TN              # 18
NCHUNK = N // 128         # 72 e-matmul chunks
SQG = 3                   # n-tiles per square group
NQ = 4                    # u-gather quarters
UQ = N // NQ              # 2304 n per quarter

USE_XGATHER = False       # X transpose via SBUF-source dma_gather (SWDGE)
USE_UGATHER = False       # U transpose via SBUF-source dma_gather (SWDGE)


def _patch_tile_drain():
    """This toolchain's walrus allows only one sync-wait per instruction.
    Split the tail drain's waits across chained drains."""
    if getattr(tile.TileContext, "_drain_patched", False):
        return

    def _drain_and_barrier_split(self, tick_clock, wait_clock):
        nc = self.nc
        drain_inst = nc.sync.drain()
        wait_clock.add_sem_waits(
            drain_inst.ins, ScopedClock({None: tick_clock.global_clock})
        )
        si = drain_inst.ins.sync_info
        if si is not None and si.on_wait and len(si.on_wait) > 1:
            extra = list(si.on_wait[1:])
            del si.on_wait[1:]
            for w in extra:
                d = nc.sync.drain()
                dsi = d.ins.sync_info
                if dsi is None:
                    d.ins.sync_info = mybir.SyncInfo(on_wait=[w], on_update=[])
                else:
                    dsi.on_wait.append(w)
        nc.all_engine_barrier()
        assert self.sems is not None
        popped = nc._tile_sem_poison_stack.pop()
        assert popped is self._sem_poison
        nc.clear_and_free_semaphores(list(self.sems.allocated().values()))
        nc.all_engine_barrier()

    tile.TileContext._drain_and_barrier = _drain_and_barrier_split
    tile.TileContext._drain_patched = True


def _split_multi_waits(nc):
    """Hoist extra sem-waits onto standalone event-sem instructions."""
    n_split = 0
    for f in nc.m.functions:
        for bb in f.blocks:
            new_list = []
            for inst in bb.instructions:
                si = inst.sync_info
                if si is not None and si.on_wait is not None and len(si.on_wait) > 1:
                    extra = list(si.on_wait[:-1])
                    keep = [si.on_wait[-1]]
                    for w in extra:
                        ev = mybir.InstEventSemaphore(
                            name=f"{inst.name}-wsplit{n_split}",
                            ins=[], outs=[],
                            sync_info=mybir.SyncInfo(on_wait=[w], on_update=[]),
                        )
                        ev.engine = inst.engine
                        nc.register_instruction(ev)
                        new_list.append(ev)
                        n_split += 1
                    del si.on_wait[:]
                    si.on_wait.extend(keep)
                new_list.append(inst)
            bb.instructions[:] = new_list
    return n_split


def _build_module():
    _patch_tile_drain()
    nc = bass.Bass()
    xin = nc.declare_dram_parameter("xin", [BPC, D, N], dt.float32, isOutput=False)
    cw = nc.declare_dram_parameter("cw", [K, D], dt.float32, isOutput=False)
    s_col = nc.declare_dram_parameter("s_col", [K, 1], dt.float32, isOutput=False)
    s_row = nc.declare_dram_parameter("s_row", [1, K], dt.float32, isOutput=False)
    gx_idx = nc.declare_dram_parameter("gx_idx", [16, 64], dt.int16, isOutput=False)
    gu_idx = nc.declare_dram_parameter("gu_idx", [16, 8], dt.int16, isOutput=False)
    eout = nc.declare_dram_parameter("eout", [BPC, K, D], dt.float32, isOutput=True)

    f16, f32 = dt.float16, dt.float32
    AX = mybir.AxisListType.X
    EXP = mybir.ActivationFunctionType.Exp

    with tile.TileContext(nc) as tc:
        with ExitStack() as ctx:
            singles = ctx.enter_context(tc.tile_pool(name="singles", bufs=1))
            psprep = ctx.enter_context(tc.tile_pool(name="psprep", bufs=1, space="PSUM"))

            # ---- one-time prep from codewords/scale ----
            cw_sb = singles.tile([K, D], f32)
            nc.sync.dma_start(cw_sb[:], cw[:])
            scol_sb = singles.tile([K, 1], f32)
            nc.sync.dma_start(scol_sb[:], s_col[:])
            srow_sb = singles.tile([1, K], f32)
            nc.sync.dma_start(srow_sb[:], s_row[:])
            gx_sb = singles.tile([16, 64], dt.int16)
            nc.sync.dma_start(gx_sb[:], gx_idx[:])
            gu_sb = singles.tile([16, 8], dt.int16)
            nc.sync.dma_start(gu_sb[:], gu_idx[:])

            # G16 [128, 2, K]: G[p, c, k] = -2 s_k c[k, c*128+p]
            w1 = singles.tile([K, D], f32)
            nc.vector.tensor_scalar_mul(w1[:], cw_sb[:], scol_sb[:])
            w2 = singles.tile([K, D], f32)
            nc.vector.tensor_scalar_mul(w2[:], w1[:], -2.0)
            w16 = singles.tile([K, D], f16)
            nc.vector.tensor_copy(w16[:], w2[:])
            g16 = singles.tile([128, 2 * K], f16)
            nc.sync.dma_start_transpose(
                g16[:].rearrange("p (c k) -> p c k", k=K), w16[:]
            )

            # S16 [128, K]: every row = s_k (fp16)
            ones_row16 = singles.tile([1, 128], f16)
            nc.vector.memset(ones_row16[:], 1.0)
            srow16 = singles.tile([1, K], f16)
            nc.vector.tensor_copy(srow16[:], srow_sb[:])
            ps_s = psprep.tile([128, K], f32)
            nc.tensor.matmul(ps_s[:], ones_row16[:], srow16[:], start=True, stop=True,
                             skip_group_check=True)
            s16 = singles.tile([128, K], f16)
            nc.vector.tensor_copy(s16[:], ps_s[:])

            # bias [K, 1] = s_k * ||c_k||^2
            csq = singles.tile([K, D], f32)
            nc.vector.tensor_mul(csq[:], cw_sb[:], cw_sb[:])
            sqc = singles.tile([K, 1], f32)
            nc.vector.reduce_sum(
                sqc[:].rearrange("k (o p) -> k o p", o=1),
                csq[:].rearrange("k (o d) -> k o d", o=1), axis=AX)
            bias = singles.tile([K, 1], f32)
            nc.vector.tensor_mul(bias[:], sqc[:], scol_sb[:])

            ones_col16 = singles.tile([128, 1], f16)
            nc.vector.memset(ones_col16[:], 1.0)

            # ---- per-batch pools ----
            xpool = ctx.enter_context(tc.tile_pool(name="x16", bufs=2))
            xtpool = ctx.enter_context(tc.tile_pool(name="xt", bufs=2))
            sqpool = ctx.enter_context(tc.tile_pool(name="xsq", bufs=2))
            upool = ctx.enter_context(tc.tile_pool(name="u16", bufs=1))
            utpool = ctx.enter_context(tc.tile_pool(name="ut", bufs=2))
            apool = ctx.enter_context(tc.tile_pool(name="a16", bufs=2))
            npool = ctx.enter_context(tc.tile_pool(name="nrm", bufs=3))
            opool = ctx.enter_context(tc.tile_pool(name="out", bufs=2))
            psl = ctx.enter_context(tc.tile_pool(name="psl", bufs=3, space="PSUM"))
            pse = ctx.enter_context(tc.tile_pool(name="pse", bufs=2, space="PSUM"))

            for b in range(BPC):
                # one SWDGE cast-load per batch: x[p, h, n] = X[b, h*128+p, n]
                x = xpool.tile([128, 2, N], f16, tag="x")
                nc.gpsimd.dma_start(
                    x[:], xin[b].rearrange("(h p) n -> p h n", p=128))
                x3 = x[:]

                # X transpose -> xt
                if USE_XGATHER:
                    # tokens = (d, n-quarter): out[p, c, i=q*256+d] =
                    #   x[d, q*2304 + c*128 + p]; rank stripe = 4608B
                    xt = xtpool.tile([128, UQ // 128, 1024], f16, tag="xt")
                    nc.gpsimd.dma_gather(
                        xt[:], x[:].rearrange("p h n -> p (h n)"), gx_sb[:],
                        num_idxs=1024, num_idxs_reg=1024, elem_size=UQ,
                        transpose=True,
                        sbuf_tokens_per_rank=128,
                        sbuf_free_dim_per_rank=UQ * 2,
                    )
                else:
                    # xbar layout: xt[p, c, j]: c<72 -> XT0[n, j], else XT1
                    xt = xtpool.tile([128, 2 * NCHUNK, 128], f16, tag="xt")
                    nc.sync.dma_start_transpose(
                        xt[:], x[:].rearrange("p h n -> p (h n)"))

                def xt_rhs(t):
                    if USE_XGATHER:
                        q, c = t // 18, t % 18
                        return (xt[:, c, q * 256:q * 256 + 128],
                                xt[:, c, q * 256 + 128:q * 256 + 256])
                    return xt[:, t, :], xt[:, NCHUNK + t, :]

                # logits + exp -> u16 [K, N]
                u16 = upool.tile([K, N], f16)
                for g in range(NT // SQG):
                    xsq = sqpool.tile([128, 2 * SQG * TN], f16, tag="xsq")
                    xsq3 = xsq[:].rearrange("p (c m) -> p c m", c=2)
                    sl = bass.ts(g, SQG * TN)
                    nc.vector.tensor_mul(xsq3[:, 0, :], x3[:, 0, sl], x3[:, 0, sl])
                    nc.vector.tensor_mul(xsq3[:, 1, :], x3[:, 1, sl], x3[:, 1, sl])
                    for j in range(SQG):
                        i = g * SQG + j
                        pl = psl.tile([K, TN], f32)
                        xs = bass.ts(i, TN)
                        js = bass.ts(j, TN)
                        g3 = g16[:].rearrange("p (c k) -> p c k", k=K)
                        nc.tensor.matmul(pl[:], g3[:, 0, :], x3[:, 0, xs],
                                         start=True, stop=False, skip_group_check=True)
                        nc.tensor.matmul(pl[:], g3[:, 1, :], x3[:, 1, xs],
                                         start=False, stop=False, skip_group_check=True)
                        nc.tensor.matmul(pl[:], s16[:], xsq3[:, 0, js],
                                         start=False, stop=False, skip_group_check=True)
                        nc.tensor.matmul(pl[:], s16[:], xsq3[:, 1, js],
                                         start=False, stop=True, skip_group_check=True)
                        nc.scalar.activation(u16[:, xs], pl[:], EXP, bias=bias[:], scale=1.0)

                # U transpose -> ut3 [128, 72, 32] (chunk order differs by path)
                if USE_UGATHER:
                    # utg[p, c, q*32+k] = u16[k, q*2304 + c*128 + p]
                    # flat view [128, 72, 32]: chunk j=(c*4+q) <-> n-chunk q*18+c
                    utg = utpool.tile([128, UQ // 128, 128], f16)
                    nc.gpsimd.dma_gather(
                        utg[:], u16[:], gu_sb[:],
                        num_idxs=128, num_idxs_reg=128, elem_size=UQ,
                        transpose=True,
                        sbuf_tokens_per_rank=32,
                        sbuf_free_dim_per_rank=UQ * 2,
                    )
                    ut3 = utg[:].rearrange("p c i -> p (c i)").rearrange(
                        "p (t k) -> p t k", k=K)

                    def achunk(t):
                        return (t % 18) * NQ + (t // 18)
                else:
                    ut = utpool.tile([128, NCHUNK * K], f16)
                    ut3 = ut[:].rearrange("p (t k) -> p t k", k=K)
                    nc.sync.dma_start_transpose(ut3, u16[:])

                    def achunk(t):
                        return t

                # normalize: A = U / sum_k U
                a16 = apool.tile([128, NCHUNK * K], f16)
                a3 = a16[:].rearrange("p (t k) -> p t k", k=K)
                NG = 4
                for g in range(NCHUNK // NG):
                    den = npool.tile([128, NG], f32, tag="den")
                    den3 = den[:].rearrange("p (t o) -> p t o", o=1)
                    sl3 = ut3[:, g * NG:(g + 1) * NG, :]
                    nc.vector.reduce_sum(den3, sl3, axis=AX)
                    rec = npool.tile([128, NG], f32, tag="rec")
                    nc.vector.reciprocal(rec[:], den[:])
                    recb = rec[:].rearrange("p (t o) -> p t o", o=1).broadcast_to((128, NG, K))
                    nc.vector.tensor_mul(a3[:, g * NG:(g + 1) * NG, :], sl3, recb)

                # E-matmul: psE[32, 257] += A_t^T @ [XT_t | ones]
                pe = pse.tile([K, 257], f32)
                for t in range(NCHUNK):
                    sp = (t == NCHUNK - 1)
                    r0, r1 = xt_rhs(t)
                    at = a3[:, achunk(t), :]
                    # start=True clears has_written for the whole PSUM bank, so
                    # only the very first matmul touching this bank may set it.
                    nc.tensor.matmul(pe[:, 0:128], at, r0,
                                     start=(t == 0), stop=sp, skip_group_check=True)
                    nc.tensor.matmul(pe[:, 128:256], at, r1,
                                     start=False, stop=sp, skip_group_check=True)
                    nc.tensor.matmul(pe[:, 256:257], at, ones_col16[:],
                                     start=False, stop=sp, skip_group_check=True)

                cs = opool.tile([K, D], f32, tag="cs")
                nc.vector.tensor_scalar_mul(cs[:], cw_sb[:], pe[:, 256:257])
                ef = opool.tile([K, D], f32, tag="ef")
                nc.vector.tensor_sub(ef[:], pe[:, 0:256], cs[:])
                nc.sync.dma_start(eout[b], ef[:])

    _split_multi_waits(nc)
    return nc


_NC_CACHE = None


def _gx_idx():
    # out column i = q*256 + d -> token idx = ((d>>7)*4 + q)*128 + (d&127);
    # unwrapped[i] = idxs[i%16, i//16]
    i = np.arange(1024, dtype=np.int32)
    q, d = i // 256, i % 256
    tok = ((d >> 7) * 4 + q) * 128 + (d & 127)
    return np.ascontiguousarray(
        tok.astype(np.int16).reshape(64, 16).T)


def _gu_idx():
    return np.ascontiguousarray(np.arange(128, dtype=np.int16).reshape(8, 16).T)


def _run(X, codewords, scale, trace=False, tmpdir=None):
    global _NC_CACHE
    if _NC_CACHE is None:
        _NC_CACHE = _build_module()
    nc = _NC_CACHE
    Xr = np.ascontiguousarray(X.reshape(B, D, N), dtype=np.float32)
    cw = np.ascontiguousarray(codewords, dtype=np.float32)
    s = np.asarray(scale, dtype=np.float32).reshape(-1)
    in_maps = []
    for c in range(NCORES):
        in_maps.append({
            "xin": Xr[c * BPC:(c + 1) * BPC],
            "cw": cw,
            "s_col": np.ascontiguousarray(s.reshape(K, 1)),
            "s_row": np.ascontiguousarray(s.reshape(1, K)),
            "gx_idx": _gx_idx(),
            "gu_idx": _gu_idx(),
        })
    kr = run_bass_kernel_spmd(nc, in_maps, list(range(NCORES)),
                              trace=trace, tmpdir=tmpdir)
    out = np.concatenate([r["eout"] for r in kr.results], axis=0)
    return out.astype(np.float32), kr


def kernel(X, codewords, scale):
    out, _ = _run(X, codewords, scale)
    return out


# revision 9
# speedup vs baseline: 1.2244x; 1.2244x over previous
"""VQ codebook soft-assignment encoding kernel for 8 trn2 NeuronCores.

Math (per batch b):
  Xf = X[b].reshape(D, N).T                        # [N, D], N = H*W
  logit[n,k] = scale[k] * (||x_n||^2 - 2<x_n,c_k> + ||c_k||^2)
  A = softmax(logit, axis=k)
  E[b,k,:] = sum_n A[n,k] * (x_n - c_k)            # [K, D]

Sharding: data-parallel over B (4 batches per core), codewords/scale replicated.

Device plan per core (all heavy compute in fp16 with fp32 PSUM accumulation):
  - SWDGE cast-load X[b] fp32 HBM -> fp16 SBUF, native [d, n] layout
  - logits in [k, n] layout:  PSUM = G^T X + S^T X^2  where G[d,k] = -2 s_k c[k,d],
    S[d,k] = s_k  (so S^T X^2 contributes s_k*||x_n||^2);  then
    U = exp(PSUM + bias_k),  bias_k = s_k ||c_k||^2  (ACT, per-partition bias)
  - DMA-xbar transposes (fp16): X -> XT [128, 72, 128] tiles ([n-part, d])
  - U -> UT via DVE stream-transpose (4x 32-partition instructions), keeping
    the scheduler's global DMA fence chain free of the U transpose
  - normalize on DVE: den = reduce_k UT, A = UT * (1/den)
  - E-matmul on PE: psE[32, 257] += A_t^T @ XT_t over 72 n-chunks
    -> cols 0:256 = sum_n A[n,k] x[n,d], col 256 = S_k = sum_n A[n,k]
  - E = psE[:, :256] - S_k * c  (DVE), DMA out fp32
"""
import numpy as np
from contextlib import ExitStack

import concourse.bass as bass
import concourse.mybir as mybir
import concourse.tile as tile
from concourse.tile import ScopedClock
from concourse.bass_utils import run_bass_kernel_spmd

dt = mybir.dt

B, D, K, H, W = 32, 256, 32, 96, 96
N = H * W                 # 9216
NCORES = 8
BPC = B // NCORES         # 4 batches per core
TN = 512                  # n-tile for logits pass
NT = N // TN              # 18
NCHUNK = N // 128         # 72 e-matmul chunks
SQG = 3                   # n-tiles per square group

USE_DVE_UT = True         # U transpose on DVE instead of DMA xbar


def _patch_tile_drain():
    """This toolchain's walrus allows only one sync-wait per instruction.
    Split the tail drain's waits across chained drains."""
    if getattr(tile.TileContext, "_drain_patched", False):
        return

    def _drain_and_barrier_split(self, tick_clock, wait_clock):
        nc = self.nc
        drain_inst = nc.sync.drain()
        wait_clock.add_sem_waits(
            drain_inst.ins, ScopedClock({None: tick_clock.global_clock})
        )
        si = drain_inst.ins.sync_info
        if si is not None and si.on_wait and len(si.on_wait) > 1:
            extra = list(si.on_wait[1:])
            del si.on_wait[1:]
            for w in extra:
                d = nc.sync.drain()
                dsi = d.ins.sync_info
                if dsi is None:
                    d.ins.sync_info = mybir.SyncInfo(on_wait=[w], on_update=[])
                else:
                    dsi.on_wait.append(w)
        nc.all_engine_barrier()
        assert self.sems is not None
        popped = nc._tile_sem_poison_stack.pop()
        assert popped is self._sem_poison
        nc.clear_and_free_semaphores(list(self.sems.allocated().values()))
        nc.all_engine_barrier()

    tile.TileContext._drain_and_barrier = _drain_and_barrier_split
    tile.TileContext._drain_patched = True


def _split_multi_waits(nc):
    """Hoist extra sem-waits onto standalone event-sem instructions."""
    n_split = 0
    for f in nc.m.functions:
        for bb in f.blocks:
            new_list = []
            for inst in bb.instructions:
                si = inst.sync_info
                if si is not None and si.on_wait is not None and len(si.on_wait) > 1:
                    extra = list(si.on_wait[:-1])
                    keep = [si.on_wait[-1]]
                    for w in extra:
                        ev = mybir.InstEventSemaphore(
                            name=f"{inst.name}-wsplit{n_split}",
                            ins=[], outs=[],
                            sync_info=mybir.SyncInfo(on_wait=[w], on_update=[]),
                        )
                        ev.engine = inst.engine
                        nc.register_instruction(ev)
                        new_list.append(ev)
                        n_split += 1
                    del si.on_wait[:]
                    si.on_wait.extend(keep)
                new_list.append(inst)
            bb.instructions[:] = new_list
    return n_split


def _build_module():
    _patch_tile_drain()
    nc = bass.Bass()
    xin = nc.declare_dram_parameter("xin", [BPC, D, N], dt.float32, isOutput=False)
    cw = nc.declare_dram_parameter("cw", [K, D], dt.float32, isOutput=False)
    s_col = nc.declare_dram_parameter("s_col", [K, 1], dt.float32, isOutput=False)
    s_row = nc.declare_dram_parameter("s_row", [1, K], dt.float32, isOutput=False)
    eout = nc.declare_dram_parameter("eout", [BPC, K, D], dt.float32, isOutput=True)

    f16, f32 = dt.float16, dt.float32
    AX = mybir.AxisListType.X
    EXP = mybir.ActivationFunctionType.Exp

    with tile.TileContext(nc) as tc:
        with ExitStack() as ctx:
            singles = ctx.enter_context(tc.tile_pool(name="singles", bufs=1))
            psprep = ctx.enter_context(tc.tile_pool(name="psprep", bufs=1, space="PSUM"))

            # ---- one-time prep from codewords/scale ----
            cw_sb = singles.tile([K, D], f32)
            nc.sync.dma_start(cw_sb[:], cw[:])
            scol_sb = singles.tile([K, 1], f32)
            nc.sync.dma_start(scol_sb[:], s_col[:])
            srow_sb = singles.tile([1, K], f32)
            nc.sync.dma_start(srow_sb[:], s_row[:])

            # G16 [128, 2, K]: G[p, c, k] = -2 s_k c[k, c*128+p]
            w1 = singles.tile([K, D], f32)
            nc.vector.tensor_scalar_mul(w1[:], cw_sb[:], scol_sb[:])
            w2 = singles.tile([K, D], f32)
            nc.vector.tensor_scalar_mul(w2[:], w1[:], -2.0)
            w16 = singles.tile([K, D], f16)
            nc.vector.tensor_copy(w16[:], w2[:])
            g16 = singles.tile([128, 2 * K], f16)
            nc.sync.dma_start_transpose(
                g16[:].rearrange("p (c k) -> p c k", k=K), w16[:]
            )

            # S16 [128, K]: every row = s_k (fp16)
            ones_row16 = singles.tile([1, 128], f16)
            nc.vector.memset(ones_row16[:], 1.0)
            srow16 = singles.tile([1, K], f16)
            nc.vector.tensor_copy(srow16[:], srow_sb[:])
            ps_s = psprep.tile([128, K], f32)
            nc.tensor.matmul(ps_s[:], ones_row16[:], srow16[:], start=True, stop=True,
                             skip_group_check=True)
            s16 = singles.tile([128, K], f16)
            nc.vector.tensor_copy(s16[:], ps_s[:])

            # bias [K, 1] = s_k * ||c_k||^2
            csq = singles.tile([K, D], f32)
            nc.vector.tensor_mul(csq[:], cw_sb[:], cw_sb[:])
            sqc = singles.tile([K, 1], f32)
            nc.vector.reduce_sum(
                sqc[:].rearrange("k (o p) -> k o p", o=1),
                csq[:].rearrange("k (o d) -> k o d", o=1), axis=AX)
            bias = singles.tile([K, 1], f32)
            nc.vector.tensor_mul(bias[:], sqc[:], scol_sb[:])

            ones_col16 = singles.tile([128, 1], f16)
            nc.vector.memset(ones_col16[:], 1.0)

            # ---- per-batch pools ----
            xpool = ctx.enter_context(tc.tile_pool(name="x16", bufs=2))
            xtpool = ctx.enter_context(tc.tile_pool(name="xt", bufs=2))
            sqpool = ctx.enter_context(tc.tile_pool(name="xsq", bufs=2))
            upool = ctx.enter_context(tc.tile_pool(name="u16", bufs=1))
            utpool = ctx.enter_context(tc.tile_pool(name="ut", bufs=2))
            apool = ctx.enter_context(tc.tile_pool(name="a16", bufs=2))
            npool = ctx.enter_context(tc.tile_pool(name="nrm", bufs=3))
            opool = ctx.enter_context(tc.tile_pool(name="out", bufs=2))
            psl = ctx.enter_context(tc.tile_pool(name="psl", bufs=3, space="PSUM"))
            pse = ctx.enter_context(tc.tile_pool(name="pse", bufs=2, space="PSUM"))

            for b in range(BPC):
                x0 = xpool.tile([128, N], f16, tag="x0")
                nc.gpsimd.dma_start(x0[:], xin[b, 0:128, :])
                x1 = xpool.tile([128, N], f16, tag="x1")
                nc.gpsimd.dma_start(x1[:], xin[b, 128:256, :])

                xt0 = xtpool.tile([128, NCHUNK * 128], f16, tag="xt0")
                xt0v = xt0[:].rearrange("p (t w) -> p t w", w=128)
                nc.sync.dma_start_transpose(xt0v, x0[:])
                xt1 = xtpool.tile([128, NCHUNK * 128], f16, tag="xt1")
                xt1v = xt1[:].rearrange("p (t w) -> p t w", w=128)
                nc.sync.dma_start_transpose(xt1v, x1[:])

                u16 = upool.tile([K, N], f16)
                for g in range(NT // SQG):
                    xsq = sqpool.tile([128, 2 * SQG * TN], f16, tag="xsq")
                    xsq3 = xsq[:].rearrange("p (c m) -> p c m", c=2)
                    sl = bass.ts(g, SQG * TN)
                    nc.vector.tensor_mul(xsq3[:, 0, :], x0[:, sl], x0[:, sl])
                    nc.vector.tensor_mul(xsq3[:, 1, :], x1[:, sl], x1[:, sl])
                    for j in range(SQG):
                        i = g * SQG + j
                        pl = psl.tile([K, TN], f32)
                        xs = bass.ts(i, TN)
                        js = bass.ts(j, TN)
                        nc.tensor.matmul(pl[:], g16[:].rearrange("p (c k) -> p c k", k=K)[:, 0, :],
                                         x0[:, xs], start=True, stop=False, skip_group_check=True)
                        nc.tensor.matmul(pl[:], g16[:].rearrange("p (c k) -> p c k", k=K)[:, 1, :],
                                         x1[:, xs], start=False, stop=False, skip_group_check=True)
                        nc.tensor.matmul(pl[:], s16[:], xsq3[:, 0, js],
                                         start=False, stop=False, skip_group_check=True)
                        nc.tensor.matmul(pl[:], s16[:], xsq3[:, 1, js],
                                         start=False, stop=True, skip_group_check=True)
                        nc.scalar.activation(u16[:, xs], pl[:], EXP, bias=bias[:], scale=1.0)

                ut = utpool.tile([128, NCHUNK * K], f16)
                ut3 = ut[:].rearrange("p (t k) -> p t k", k=K)
                if USE_DVE_UT:
                    # ut3[32j+w, t, k] = u16[k, t*128 + 32j + w]:
                    # 4 DVE stream-transposes (32x32 blocks), one per j
                    uv = u16[:].rearrange("k (t j w) -> k t j w", j=4, w=32)
                    for j in range(4):
                        nc.vector.transpose(
                            ut3[32 * j:32 * (j + 1), :, :], uv[:, :, j, :])
                else:
                    nc.sync.dma_start_transpose(ut3, u16[:])

                a16 = apool.tile([128, NCHUNK * K], f16)
                a3 = a16[:].rearrange("p (t k) -> p t k", k=K)
                NG = 4
                for g in range(NCHUNK // NG):
                    den = npool.tile([128, NG], f32, tag="den")
                    den3 = den[:].rearrange("p (t o) -> p t o", o=1)
                    sl3 = ut3[:, g * NG:(g + 1) * NG, :]
                    nc.vector.reduce_sum(den3, sl3, axis=AX)
                    rec = npool.tile([128, NG], f32, tag="rec")
                    nc.vector.reciprocal(rec[:], den[:])
                    recb = rec[:].rearrange("p (t o) -> p t o", o=1).broadcast_to((128, NG, K))
                    nc.vector.tensor_mul(a3[:, g * NG:(g + 1) * NG, :], sl3, recb)

                pe = pse.tile([K, 257], f32)
                for t in range(NCHUNK):
                    sp = (t == NCHUNK - 1)
                    # start=True clears has_written for the whole PSUM bank, so
                    # only the very first matmul touching this bank may set it.
                    nc.tensor.matmul(pe[:, 0:128], a3[:, t, :], xt0v[:, t, :],
                                     start=(t == 0), stop=sp, skip_group_check=True)
                    nc.tensor.matmul(pe[:, 128:256], a3[:, t, :], xt1v[:, t, :],
                                     start=False, stop=sp, skip_group_check=True)
                    nc.tensor.matmul(pe[:, 256:257], a3[:, t, :], ones_col16[:],
                                     start=False, stop=sp, skip_group_check=True)

                cs = opool.tile([K, D], f32, tag="cs")
                nc.vector.tensor_scalar_mul(cs[:], cw_sb[:], pe[:, 256:257])
                ef = opool.tile([K, D], f32, tag="ef")
                nc.vector.tensor_sub(ef[:], pe[:, 0:256], cs[:])
                nc.sync.dma_start(eout[b], ef[:])

    _split_multi_waits(nc)
    return nc


_NC_CACHE = None


def _run(X, codewords, scale, trace=False, tmpdir=None):
    global _NC_CACHE
    if _NC_CACHE is None:
        _NC_CACHE = _build_module()
    nc = _NC_CACHE
    Xr = np.ascontiguousarray(X.reshape(B, D, N), dtype=np.float32)
    cw = np.ascontiguousarray(codewords, dtype=np.float32)
    s = np.asarray(scale, dtype=np.float32).reshape(-1)
    in_maps = []
    for c in range(NCORES):
        in_maps.append({
            "xin": Xr[c * BPC:(c + 1) * BPC],
            "cw": cw,
            "s_col": np.ascontiguousarray(s.reshape(K, 1)),
            "s_row": np.ascontiguousarray(s.reshape(1, K)),
        })
    kr = run_bass_kernel_spmd(nc, in_maps, list(range(NCORES)),
                              trace=trace, tmpdir=tmpdir)
    out = np.concatenate([r["eout"] for r in kr.results], axis=0)
    return out.astype(np.float32), kr


def kernel(X, codewords, scale):
    out, _ = _run(X, codewords, scale)
    return out


# revision 10
# speedup vs baseline: 1.2658x; 1.0338x over previous
"""VQ codebook soft-assignment encoding kernel for 8 trn2 NeuronCores.

Math (per batch b):
  Xf = X[b].reshape(D, N).T                        # [N, D], N = H*W
  logit[n,k] = scale[k] * (||x_n||^2 - 2<x_n,c_k> + ||c_k||^2)
  A = softmax(logit, axis=k)
  E[b,k,:] = sum_n A[n,k] * (x_n - c_k)            # [K, D]

Sharding: data-parallel over B (4 batches per core), codewords/scale replicated.

Device plan per core (all heavy compute in fp16 with fp32 PSUM accumulation):
  - SWDGE cast-load X[b] fp32 HBM -> fp16 SBUF, native [d, n] layout
  - logits in [k, n] layout:  PSUM = G^T X + S^T X^2  where G[d,k] = -2 s_k c[k,d],
    S[d,k] = s_k  (so S^T X^2 contributes s_k*||x_n||^2);  then
    U = exp(PSUM + bias_k),  bias_k = s_k ||c_k||^2  (ACT, per-partition bias)
  - DMA-xbar transposes (fp16): X -> XT [128, 72, 128] tiles ([n-part, d])
  - U -> UT via DVE stream-transpose (4x 32-partition instructions), keeping
    the scheduler's global DMA fence chain free of the U transpose
  - normalize on DVE: den = reduce_k UT, A = UT * (1/den)
  - E-matmul on PE: psE[32, 257] += A_t^T @ XT_t over 72 n-chunks
    -> cols 0:256 = sum_n A[n,k] x[n,d], col 256 = S_k = sum_n A[n,k]
  - E = psE[:, :256] - S_k * c  (DVE), DMA out fp32
"""
import numpy as np
from contextlib import ExitStack

import concourse.bass as bass
import concourse.mybir as mybir
import concourse.tile as tile
from concourse.tile import ScopedClock
from concourse.bass_utils import run_bass_kernel_spmd

dt = mybir.dt

B, D, K, H, W = 32, 256, 32, 96, 96
N = H * W                 # 9216
NCORES = 8
BPC = B // NCORES         # 4 batches per core
TN = 512                  # n-tile for logits pass
NT = N // TN              # 18
NCHUNK = N // 128         # 72 e-matmul chunks
SQG = 3                   # n-tiles per square group

USE_DVE_UT = True         # U transpose on DVE instead of DMA xbar


def _patch_tile_drain():
    """This toolchain's walrus allows only one sync-wait per instruction.
    Split the tail drain's waits across chained drains."""
    if getattr(tile.TileContext, "_drain_patched", False):
        return

    def _drain_and_barrier_split(self, tick_clock, wait_clock):
        nc = self.nc
        drain_inst = nc.sync.drain()
        wait_clock.add_sem_waits(
            drain_inst.ins, ScopedClock({None: tick_clock.global_clock})
        )
        si = drain_inst.ins.sync_info
        if si is not None and si.on_wait and len(si.on_wait) > 1:
            extra = list(si.on_wait[1:])
            del si.on_wait[1:]
            for w in extra:
                d = nc.sync.drain()
                dsi = d.ins.sync_info
                if dsi is None:
                    d.ins.sync_info = mybir.SyncInfo(on_wait=[w], on_update=[])
                else:
                    dsi.on_wait.append(w)
        nc.all_engine_barrier()
        assert self.sems is not None
        popped = nc._tile_sem_poison_stack.pop()
        assert popped is self._sem_poison
        nc.clear_and_free_semaphores(list(self.sems.allocated().values()))
        nc.all_engine_barrier()

    tile.TileContext._drain_and_barrier = _drain_and_barrier_split
    tile.TileContext._drain_patched = True


def _split_multi_waits(nc):
    """Hoist extra sem-waits onto standalone event-sem instructions."""
    n_split = 0
    for f in nc.m.functions:
        for bb in f.blocks:
            new_list = []
            for inst in bb.instructions:
                si = inst.sync_info
                if si is not None and si.on_wait is not None and len(si.on_wait) > 1:
                    extra = list(si.on_wait[:-1])
                    keep = [si.on_wait[-1]]
                    for w in extra:
                        ev = mybir.InstEventSemaphore(
                            name=f"{inst.name}-wsplit{n_split}",
                            ins=[], outs=[],
                            sync_info=mybir.SyncInfo(on_wait=[w], on_update=[]),
                        )
                        ev.engine = inst.engine
                        nc.register_instruction(ev)
                        new_list.append(ev)
                        n_split += 1
                    del si.on_wait[:]
                    si.on_wait.extend(keep)
                new_list.append(inst)
            bb.instructions[:] = new_list
    return n_split


def _build_module():
    _patch_tile_drain()
    nc = bass.Bass()
    xin = nc.declare_dram_parameter("xin", [BPC, D, N], dt.float32, isOutput=False)
    cw = nc.declare_dram_parameter("cw", [K, D], dt.float32, isOutput=False)
    s_col = nc.declare_dram_parameter("s_col", [K, 1], dt.float32, isOutput=False)
    s_row = nc.declare_dram_parameter("s_row", [1, K], dt.float32, isOutput=False)
    eout = nc.declare_dram_parameter("eout", [BPC, K, D], dt.float32, isOutput=True)

    f16, f32 = dt.float16, dt.float32
    AX = mybir.AxisListType.X
    EXP = mybir.ActivationFunctionType.Exp

    with tile.TileContext(nc) as tc:
        with ExitStack() as ctx:
            singles = ctx.enter_context(tc.tile_pool(name="singles", bufs=1))
            psprep = ctx.enter_context(tc.tile_pool(name="psprep", bufs=1, space="PSUM"))

            # ---- one-time prep from codewords/scale ----
            cw_sb = singles.tile([K, D], f32)
            nc.sync.dma_start(cw_sb[:], cw[:])
            scol_sb = singles.tile([K, 1], f32)
            nc.sync.dma_start(scol_sb[:], s_col[:])
            srow_sb = singles.tile([1, K], f32)
            nc.sync.dma_start(srow_sb[:], s_row[:])

            # G16 [128, 2, K]: G[p, c, k] = -2 s_k c[k, c*128+p]
            w1 = singles.tile([K, D], f32)
            nc.vector.tensor_scalar_mul(w1[:], cw_sb[:], scol_sb[:])
            w2 = singles.tile([K, D], f32)
            nc.vector.tensor_scalar_mul(w2[:], w1[:], -2.0)
            w16 = singles.tile([K, D], f16)
            nc.vector.tensor_copy(w16[:], w2[:])
            g16 = singles.tile([128, 2 * K], f16)
            nc.sync.dma_start_transpose(
                g16[:].rearrange("p (c k) -> p c k", k=K), w16[:]
            )

            # S16 [128, K]: every row = s_k (fp16)
            ones_row16 = singles.tile([1, 128], f16)
            nc.vector.memset(ones_row16[:], 1.0)
            srow16 = singles.tile([1, K], f16)
            nc.vector.tensor_copy(srow16[:], srow_sb[:])
            ps_s = psprep.tile([128, K], f32)
            nc.tensor.matmul(ps_s[:], ones_row16[:], srow16[:], start=True, stop=True,
                             skip_group_check=True)
            s16 = singles.tile([128, K], f16)
            nc.vector.tensor_copy(s16[:], ps_s[:])

            # bias [K, 1] = s_k * ||c_k||^2
            csq = singles.tile([K, D], f32)
            nc.vector.tensor_mul(csq[:], cw_sb[:], cw_sb[:])
            sqc = singles.tile([K, 1], f32)
            nc.vector.reduce_sum(
                sqc[:].rearrange("k (o p) -> k o p", o=1),
                csq[:].rearrange("k (o d) -> k o d", o=1), axis=AX)
            bias = singles.tile([K, 1], f32)
            nc.vector.tensor_mul(bias[:], sqc[:], scol_sb[:])

            ones_col16 = singles.tile([128, 1], f16)
            nc.vector.memset(ones_col16[:], 1.0)

            # ---- per-batch pools ----
            xpool = ctx.enter_context(tc.tile_pool(name="x16", bufs=2))
            xtpool = ctx.enter_context(tc.tile_pool(name="xt", bufs=2))
            sqpool = ctx.enter_context(tc.tile_pool(name="xsq", bufs=2))
            upool = ctx.enter_context(tc.tile_pool(name="u16", bufs=1))
            utpool = ctx.enter_context(tc.tile_pool(name="ut", bufs=2))
            apool = ctx.enter_context(tc.tile_pool(name="a16", bufs=2))
            npool = ctx.enter_context(tc.tile_pool(name="nrm", bufs=3))
            opool = ctx.enter_context(tc.tile_pool(name="out", bufs=2))
            psl = ctx.enter_context(tc.tile_pool(name="psl", bufs=3, space="PSUM"))
            pse = ctx.enter_context(tc.tile_pool(name="pse", bufs=2, space="PSUM"))

            for b in range(BPC):
                x0 = xpool.tile([128, N], f16, tag="x0")
                nc.gpsimd.dma_start(x0[:], xin[b, 0:128, :])
                x1 = xpool.tile([128, N], f16, tag="x1")
                nc.gpsimd.dma_start(x1[:], xin[b, 128:256, :])

                xt0 = xtpool.tile([128, NCHUNK * 128], f16, tag="xt0")
                xt0v = xt0[:].rearrange("p (t w) -> p t w", w=128)
                nc.sync.dma_start_transpose(xt0v, x0[:])
                xt1 = xtpool.tile([128, NCHUNK * 128], f16, tag="xt1")
                xt1v = xt1[:].rearrange("p (t w) -> p t w", w=128)
                nc.sync.dma_start_transpose(xt1v, x1[:])

                u16 = upool.tile([K, N], f16)
                for g in range(NT // SQG):
                    xsq = sqpool.tile([128, 2 * SQG * TN], f16, tag="xsq")
                    xsq3 = xsq[:].rearrange("p (c m) -> p c m", c=2)
                    sl = bass.ts(g, SQG * TN)
                    nc.vector.tensor_mul(xsq3[:, 0, :], x0[:, sl], x0[:, sl])
                    nc.vector.tensor_mul(xsq3[:, 1, :], x1[:, sl], x1[:, sl])
                    for j in range(SQG):
                        i = g * SQG + j
                        pl = psl.tile([K, TN], f32)
                        xs = bass.ts(i, TN)
                        js = bass.ts(j, TN)
                        nc.tensor.matmul(pl[:], g16[:].rearrange("p (c k) -> p c k", k=K)[:, 0, :],
                                         x0[:, xs], start=True, stop=False, skip_group_check=True)
                        nc.tensor.matmul(pl[:], g16[:].rearrange("p (c k) -> p c k", k=K)[:, 1, :],
                                         x1[:, xs], start=False, stop=False, skip_group_check=True)
                        nc.tensor.matmul(pl[:], s16[:], xsq3[:, 0, js],
                                         start=False, stop=False, skip_group_check=True)
                        nc.tensor.matmul(pl[:], s16[:], xsq3[:, 1, js],
                                         start=False, stop=True, skip_group_check=True)
                        nc.scalar.activation(u16[:, xs], pl[:], EXP, bias=bias[:], scale=1.0)

                ut = utpool.tile([128, NCHUNK * K], f16)
                ut3 = ut[:].rearrange("p (t k) -> p t k", k=K)
                if USE_DVE_UT:
                    # ut3[32j+w, t, k] = u16[k, t*128 + 32j + w]:
                    # 4 DVE stream-transposes (32x32 blocks), one per j
                    uv = u16[:].rearrange("k (t j w) -> k t j w", j=4, w=32)
                    for j in range(4):
                        nc.vector.transpose(
                            ut3[32 * j:32 * (j + 1), :, :], uv[:, :, j, :])
                else:
                    nc.sync.dma_start_transpose(ut3, u16[:])

                # per-group A tiles so E-matmuls start after the first
                # normalize group instead of after all 72 chunks
                NG = 4
                pe = pse.tile([K, 257], f32)
                for g in range(NCHUNK // NG):
                    den = npool.tile([128, NG], f32, tag="den")
                    den3 = den[:].rearrange("p (t o) -> p t o", o=1)
                    sl3 = ut3[:, g * NG:(g + 1) * NG, :]
                    nc.vector.reduce_sum(den3, sl3, axis=AX)
                    rec = npool.tile([128, NG], f32, tag="rec")
                    nc.vector.reciprocal(rec[:], den[:])
                    recb = rec[:].rearrange("p (t o) -> p t o", o=1).broadcast_to((128, NG, K))
                    ag = apool.tile([128, NG * K], f16, tag=f"a{g}")
                    ag3 = ag[:].rearrange("p (t k) -> p t k", k=K)
                    nc.vector.tensor_mul(ag3[:], sl3, recb)
                    for tt in range(NG):
                        t = g * NG + tt
                        sp = (t == NCHUNK - 1)
                        # start=True clears has_written for the whole PSUM
                        # bank, so only the very first matmul touching this
                        # bank may set it.
                        nc.tensor.matmul(pe[:, 0:128], ag3[:, tt, :], xt0v[:, t, :],
                                         start=(t == 0), stop=sp, skip_group_check=True)
                        nc.tensor.matmul(pe[:, 128:256], ag3[:, tt, :], xt1v[:, t, :],
                                         start=False, stop=sp, skip_group_check=True)
                        nc.tensor.matmul(pe[:, 256:257], ag3[:, tt, :], ones_col16[:],
                                         start=False, stop=sp, skip_group_check=True)

                cs = opool.tile([K, D], f32, tag="cs")
                nc.vector.tensor_scalar_mul(cs[:], cw_sb[:], pe[:, 256:257])
                ef = opool.tile([K, D], f32, tag="ef")
                nc.vector.tensor_sub(ef[:], pe[:, 0:256], cs[:])
                nc.sync.dma_start(eout[b], ef[:])

    _split_multi_waits(nc)
    return nc


_NC_CACHE = None


def _run(X, codewords, scale, trace=False, tmpdir=None):
    global _NC_CACHE
    if _NC_CACHE is None:
        _NC_CACHE = _build_module()
    nc = _NC_CACHE
    Xr = np.ascontiguousarray(X.reshape(B, D, N), dtype=np.float32)
    cw = np.ascontiguousarray(codewords, dtype=np.float32)
    s = np.asarray(scale, dtype=np.float32).reshape(-1)
    in_maps = []
    for c in range(NCORES):
        in_maps.append({
            "xin": Xr[c * BPC:(c + 1) * BPC],
            "cw": cw,
            "s_col": np.ascontiguousarray(s.reshape(K, 1)),
            "s_row": np.ascontiguousarray(s.reshape(1, K)),
        })
    kr = run_bass_kernel_spmd(nc, in_maps, list(range(NCORES)),
                              trace=trace, tmpdir=tmpdir)
    out = np.concatenate([r["eout"] for r in kr.results], axis=0)
    return out.astype(np.float32), kr


def kernel(X, codewords, scale):
    out, _ = _run(X, codewords, scale)
    return out


# revision 13
# speedup vs baseline: 1.2859x; 1.0159x over previous
"""VQ codebook soft-assignment encoding kernel for 8 trn2 NeuronCores.

Math (per batch b):
  Xf = X[b].reshape(D, N).T                        # [N, D], N = H*W
  logit[n,k] = scale[k] * (||x_n||^2 - 2<x_n,c_k> + ||c_k||^2)
  A = softmax(logit, axis=k)
  E[b,k,:] = sum_n A[n,k] * (x_n - c_k)            # [K, D]

Sharding: data-parallel over B (4 batches per core), codewords/scale replicated.

Device plan per core (all heavy compute in fp16 with fp32 PSUM accumulation):
  - SWDGE cast-load X[b] fp32 HBM -> fp16 SBUF, native [d, n] layout
  - logits in [k, n] layout:  PSUM = G^T X + S^T X^2  where G[d,k] = -2 s_k c[k,d],
    S[d,k] = s_k  (so S^T X^2 contributes s_k*||x_n||^2);  then
    U = exp(PSUM + bias_k),  bias_k = s_k ||c_k||^2  (ACT, per-partition bias)
  - DMA-xbar transposes (fp16): X -> XT [128, 72, 128] tiles ([n-part, d])
  - U -> UT via DVE stream-transpose (4x 32-partition instructions), keeping
    the scheduler's global DMA fence chain free of the U transpose
  - normalize on DVE: den = reduce_k UT, A = UT * (1/den)
  - E-matmul on PE: psE[32, 257] += A_t^T @ XT_t over 72 n-chunks
    -> cols 0:256 = sum_n A[n,k] x[n,d], col 256 = S_k = sum_n A[n,k]
  - E = psE[:, :256] - S_k * c  (DVE), DMA out fp32
"""
import numpy as np
from contextlib import ExitStack

import concourse.bass as bass
import concourse.mybir as mybir
import concourse.tile as tile
from concourse.tile import ScopedClock
from concourse.bass_utils import run_bass_kernel_spmd

dt = mybir.dt

B, D, K, H, W = 32, 256, 32, 96, 96
N = H * W                 # 9216
NCORES = 8
BPC = B // NCORES         # 4 batches per core
TN = 512                  # n-tile for logits pass
NT = N // TN              # 18
NCHUNK = N // 128         # 72 e-matmul chunks
SQG = 3                   # n-tiles per square group

USE_DVE_UT = True         # U transpose on DVE instead of DMA xbar


def _patch_tile_drain():
    """This toolchain's walrus allows only one sync-wait per instruction.
    Split the tail drain's waits across chained drains."""
    if getattr(tile.TileContext, "_drain_patched", False):
        return

    def _drain_and_barrier_split(self, tick_clock, wait_clock):
        nc = self.nc
        drain_inst = nc.sync.drain()
        wait_clock.add_sem_waits(
            drain_inst.ins, ScopedClock({None: tick_clock.global_clock})
        )
        si = drain_inst.ins.sync_info
        if si is not None and si.on_wait and len(si.on_wait) > 1:
            extra = list(si.on_wait[1:])
            del si.on_wait[1:]
            for w in extra:
                d = nc.sync.drain()
                dsi = d.ins.sync_info
                if dsi is None:
                    d.ins.sync_info = mybir.SyncInfo(on_wait=[w], on_update=[])
                else:
                    dsi.on_wait.append(w)
        nc.all_engine_barrier()
        assert self.sems is not None
        popped = nc._tile_sem_poison_stack.pop()
        assert popped is self._sem_poison
        nc.clear_and_free_semaphores(list(self.sems.allocated().values()))
        nc.all_engine_barrier()

    tile.TileContext._drain_and_barrier = _drain_and_barrier_split
    tile.TileContext._drain_patched = True


def _split_multi_waits(nc):
    """Hoist extra sem-waits onto standalone event-sem instructions."""
    n_split = 0
    for f in nc.m.functions:
        for bb in f.blocks:
            new_list = []
            for inst in bb.instructions:
                si = inst.sync_info
                if si is not None and si.on_wait is not None and len(si.on_wait) > 1:
                    extra = list(si.on_wait[:-1])
                    keep = [si.on_wait[-1]]
                    for w in extra:
                        ev = mybir.InstEventSemaphore(
                            name=f"{inst.name}-wsplit{n_split}",
                            ins=[], outs=[],
                            sync_info=mybir.SyncInfo(on_wait=[w], on_update=[]),
                        )
                        ev.engine = inst.engine
                        nc.register_instruction(ev)
                        new_list.append(ev)
                        n_split += 1
                    del si.on_wait[:]
                    si.on_wait.extend(keep)
                new_list.append(inst)
            bb.instructions[:] = new_list
    return n_split


def _build_module():
    _patch_tile_drain()
    nc = bass.Bass()
    xin = nc.declare_dram_parameter("xin", [BPC, D, N], dt.float32, isOutput=False)
    cw = nc.declare_dram_parameter("cw", [K, D], dt.float32, isOutput=False)
    s_col = nc.declare_dram_parameter("s_col", [K, 1], dt.float32, isOutput=False)
    s_row = nc.declare_dram_parameter("s_row", [1, K], dt.float32, isOutput=False)
    eout = nc.declare_dram_parameter("eout", [BPC, K, D], dt.float32, isOutput=True)

    f16, f32 = dt.float16, dt.float32
    AX = mybir.AxisListType.X
    EXP = mybir.ActivationFunctionType.Exp

    with tile.TileContext(nc) as tc:
        with ExitStack() as ctx:
            singles = ctx.enter_context(tc.tile_pool(name="singles", bufs=1))
            psprep = ctx.enter_context(tc.tile_pool(name="psprep", bufs=1, space="PSUM"))

            # ---- one-time prep from codewords/scale ----
            cw_sb = singles.tile([K, D], f32)
            nc.sync.dma_start(cw_sb[:], cw[:])
            scol_sb = singles.tile([K, 1], f32)
            nc.sync.dma_start(scol_sb[:], s_col[:])
            srow_sb = singles.tile([1, K], f32)
            nc.sync.dma_start(srow_sb[:], s_row[:])

            # G16 [128, 2, K]: G[p, c, k] = -2 s_k c[k, c*128+p]
            w1 = singles.tile([K, D], f32)
            nc.vector.tensor_scalar_mul(w1[:], cw_sb[:], scol_sb[:])
            w2 = singles.tile([K, D], f32)
            nc.vector.tensor_scalar_mul(w2[:], w1[:], -2.0)
            w16 = singles.tile([K, D], f16)
            nc.vector.tensor_copy(w16[:], w2[:])
            g16 = singles.tile([128, 2 * K], f16)
            nc.sync.dma_start_transpose(
                g16[:].rearrange("p (c k) -> p c k", k=K), w16[:]
            )

            # S16 [128, K]: every row = s_k (fp16)
            ones_row16 = singles.tile([1, 128], f16)
            nc.vector.memset(ones_row16[:], 1.0)
            srow16 = singles.tile([1, K], f16)
            nc.vector.tensor_copy(srow16[:], srow_sb[:])
            ps_s = psprep.tile([128, K], f32)
            nc.tensor.matmul(ps_s[:], ones_row16[:], srow16[:], start=True, stop=True,
                             skip_group_check=True)
            s16 = singles.tile([128, K], f16)
            nc.vector.tensor_copy(s16[:], ps_s[:])

            # bias [K, 1] = s_k * ||c_k||^2
            csq = singles.tile([K, D], f32)
            nc.vector.tensor_mul(csq[:], cw_sb[:], cw_sb[:])
            sqc = singles.tile([K, 1], f32)
            nc.vector.reduce_sum(
                sqc[:].rearrange("k (o p) -> k o p", o=1),
                csq[:].rearrange("k (o d) -> k o d", o=1), axis=AX)
            bias = singles.tile([K, 1], f32)
            nc.vector.tensor_mul(bias[:], sqc[:], scol_sb[:])

            ones_col16 = singles.tile([128, 1], f16)
            nc.vector.memset(ones_col16[:], 1.0)

            # ---- per-batch pools ----
            xpool = ctx.enter_context(tc.tile_pool(name="x16", bufs=2))
            xtpool = ctx.enter_context(tc.tile_pool(name="xt", bufs=2))
            sqpool = ctx.enter_context(tc.tile_pool(name="xsq", bufs=2))
            upool = ctx.enter_context(tc.tile_pool(name="u16", bufs=1))
            utpool = ctx.enter_context(tc.tile_pool(name="ut", bufs=2))
            apool = ctx.enter_context(tc.tile_pool(name="a16", bufs=2))
            npool = ctx.enter_context(tc.tile_pool(name="nrm", bufs=3))
            opool = ctx.enter_context(tc.tile_pool(name="out", bufs=2))
            psl = ctx.enter_context(tc.tile_pool(name="psl", bufs=3, space="PSUM"))
            pse = ctx.enter_context(tc.tile_pool(name="pse", bufs=2, space="PSUM"))

            for b in range(BPC):
                x0 = xpool.tile([128, N], f16, tag="x0")
                nc.gpsimd.dma_start(x0[:], xin[b, 0:128, :])
                x1 = xpool.tile([128, N], f16, tag="x1")
                nc.gpsimd.dma_start(x1[:], xin[b, 128:256, :])

                xt0 = xtpool.tile([128, NCHUNK * 128], f16, tag="xt0")
                xt0v = xt0[:].rearrange("p (t w) -> p t w", w=128)
                nc.sync.dma_start_transpose(xt0v, x0[:])
                xt1 = xtpool.tile([128, NCHUNK * 128], f16, tag="xt1")
                xt1v = xt1[:].rearrange("p (t w) -> p t w", w=128)
                nc.sync.dma_start_transpose(xt1v, x1[:])

                # U in two half tiles so the DVE transpose of half A can
                # start while exp tiles of half B are still being produced
                u16a = upool.tile([K, N // 2], f16, tag="ua")
                u16b = upool.tile([K, N // 2], f16, tag="ub")

                def u_slice(i):
                    lo = i * TN
                    if lo < N // 2:
                        return u16a[:, lo:lo + TN]
                    return u16b[:, lo - N // 2:lo - N // 2 + TN]

                for g in range(NT // SQG):
                    xsq = sqpool.tile([128, 2 * SQG * TN], f16, tag="xsq")
                    xsq3 = xsq[:].rearrange("p (c m) -> p c m", c=2)
                    sl = bass.ts(g, SQG * TN)
                    nc.vector.tensor_mul(xsq3[:, 0, :], x0[:, sl], x0[:, sl])
                    nc.vector.tensor_mul(xsq3[:, 1, :], x1[:, sl], x1[:, sl])
                    for j in range(SQG):
                        i = g * SQG + j
                        pl = psl.tile([K, TN], f32)
                        xs = bass.ts(i, TN)
                        js = bass.ts(j, TN)
                        nc.tensor.matmul(pl[:], g16[:].rearrange("p (c k) -> p c k", k=K)[:, 0, :],
                                         x0[:, xs], start=True, stop=False, skip_group_check=True)
                        nc.tensor.matmul(pl[:], g16[:].rearrange("p (c k) -> p c k", k=K)[:, 1, :],
                                         x1[:, xs], start=False, stop=False, skip_group_check=True)
                        nc.tensor.matmul(pl[:], s16[:], xsq3[:, 0, js],
                                         start=False, stop=False, skip_group_check=True)
                        nc.tensor.matmul(pl[:], s16[:], xsq3[:, 1, js],
                                         start=False, stop=True, skip_group_check=True)
                        nc.scalar.activation(u_slice(i), pl[:], EXP, bias=bias[:], scale=1.0)

                ut = utpool.tile([128, NCHUNK * K], f16)
                ut3 = ut[:].rearrange("p (t k) -> p t k", k=K)
                # ut3[32j+w, t, k] = U[k, t*128 + 32j + w]:
                # 32x32-block DVE stream-transposes, per j and per U-half
                HT = NCHUNK // 2
                for uh, ut16 in ((0, u16a), (1, u16b)):
                    uv = ut16[:].rearrange("k (t j w) -> k t j w", j=4, w=32)
                    for j in range(4):
                        nc.vector.transpose(
                            ut3[32 * j:32 * (j + 1), uh * HT:(uh + 1) * HT, :],
                            uv[:, :, j, :])

                # per-group A tiles so E-matmuls start after the first
                # normalize group instead of after all 72 chunks
                NG = 4
                pe = pse.tile([K, 257], f32)
                for g in range(NCHUNK // NG):
                    den = npool.tile([128, NG], f32, tag="den")
                    den3 = den[:].rearrange("p (t o) -> p t o", o=1)
                    sl3 = ut3[:, g * NG:(g + 1) * NG, :]
                    nc.vector.reduce_sum(den3, sl3, axis=AX)
                    rec = npool.tile([128, NG], f32, tag="rec")
                    nc.vector.reciprocal(rec[:], den[:])
                    recb = rec[:].rearrange("p (t o) -> p t o", o=1).broadcast_to((128, NG, K))
                    ag = apool.tile([128, NG * K], f16, tag=f"a{g}")
                    ag3 = ag[:].rearrange("p (t k) -> p t k", k=K)
                    nc.vector.tensor_mul(ag3[:], sl3, recb)
                    for tt in range(NG):
                        t = g * NG + tt
                        sp = (t == NCHUNK - 1)
                        # start=True clears has_written for the whole PSUM
                        # bank, so only the very first matmul touching this
                        # bank may set it.
                        nc.tensor.matmul(pe[:, 0:128], ag3[:, tt, :], xt0v[:, t, :],
                                         start=(t == 0), stop=sp, skip_group_check=True)
                        nc.tensor.matmul(pe[:, 128:256], ag3[:, tt, :], xt1v[:, t, :],
                                         start=False, stop=sp, skip_group_check=True)
                        nc.tensor.matmul(pe[:, 256:257], ag3[:, tt, :], ones_col16[:],
                                         start=False, stop=sp, skip_group_check=True)

                cs = opool.tile([K, D], f32, tag="cs")
                nc.vector.tensor_scalar_mul(cs[:], cw_sb[:], pe[:, 256:257])
                ef = opool.tile([K, D], f32, tag="ef")
                nc.vector.tensor_sub(ef[:], pe[:, 0:256], cs[:])
                nc.sync.dma_start(eout[b], ef[:])

    _split_multi_waits(nc)
    return nc


_NC_CACHE = None


def _run(X, codewords, scale, trace=False, tmpdir=None):
    global _NC_CACHE
    if _NC_CACHE is None:
        _NC_CACHE = _build_module()
    nc = _NC_CACHE
    Xr = np.ascontiguousarray(X.reshape(B, D, N), dtype=np.float32)
    cw = np.ascontiguousarray(codewords, dtype=np.float32)
    s = np.asarray(scale, dtype=np.float32).reshape(-1)
    in_maps = []
    for c in range(NCORES):
        in_maps.append({
            "xin": Xr[c * BPC:(c + 1) * BPC],
            "cw": cw,
            "s_col": np.ascontiguousarray(s.reshape(K, 1)),
            "s_row": np.ascontiguousarray(s.reshape(1, K)),
        })
    kr = run_bass_kernel_spmd(nc, in_maps, list(range(NCORES)),
                              trace=trace, tmpdir=tmpdir)
    out = np.concatenate([r["eout"] for r in kr.results], axis=0)
    return out.astype(np.float32), kr


def kernel(X, codewords, scale):
    out, _ = _run(X, codewords, scale)
    return out


# revision 17
# speedup vs baseline: 1.3121x; 1.0204x over previous
"""VQ codebook soft-assignment encoding kernel for 8 trn2 NeuronCores.

Math (per batch b):
  Xf = X[b].reshape(D, N).T                        # [N, D], N = H*W
  logit[n,k] = scale[k] * (||x_n||^2 - 2<x_n,c_k> + ||c_k||^2)
  A = softmax(logit, axis=k)
  E[b,k,:] = sum_n A[n,k] * (x_n - c_k)            # [K, D]

Sharding: data-parallel over B (4 batches per core), codewords/scale replicated.

Device plan per core (all heavy compute in fp16 with fp32 PSUM accumulation):
  - SWDGE cast-load X[b] fp32 HBM -> fp16 SBUF, native [d, n] layout
  - logits in [k, n] layout:  PSUM = G^T X + S^T X^2  where G[d,k] = -2 s_k c[k,d],
    S[d,k] = s_k  (so S^T X^2 contributes s_k*||x_n||^2);  then
    U = exp(PSUM + bias_k),  bias_k = s_k ||c_k||^2  (ACT, per-partition bias)
  - DMA-xbar transposes (fp16): X -> XT [128, 72, 128] tiles ([n-part, d])
  - U -> UT via DVE stream-transpose (4x 32-partition instructions), keeping
    the scheduler's global DMA fence chain free of the U transpose
  - normalize on DVE: den = reduce_k UT, A = UT * (1/den)
  - E-matmul on PE: psE[32, 257] += A_t^T @ XT_t over 72 n-chunks
    -> cols 0:256 = sum_n A[n,k] x[n,d], col 256 = S_k = sum_n A[n,k]
  - E = psE[:, :256] - S_k * c  (DVE), DMA out fp32
"""
import numpy as np
from contextlib import ExitStack

import concourse.bass as bass
import concourse.mybir as mybir
import concourse.tile as tile
from concourse.tile import ScopedClock
from concourse.bass_utils import run_bass_kernel_spmd

dt = mybir.dt

B, D, K, H, W = 32, 256, 32, 96, 96
N = H * W                 # 9216
NCORES = 8
BPC = B // NCORES         # 4 batches per core
TN = 512                  # n-tile for logits pass
NT = N // TN              # 18
NCHUNK = N // 128         # 72 e-matmul chunks
SQG = 3                   # n-tiles per square group

USE_DVE_UT = True         # U transpose on DVE instead of DMA xbar


def _patch_tile_drain():
    """This toolchain's walrus allows only one sync-wait per instruction.
    Split the tail drain's waits across chained drains."""
    if getattr(tile.TileContext, "_drain_patched", False):
        return

    def _drain_and_barrier_split(self, tick_clock, wait_clock):
        nc = self.nc
        drain_inst = nc.sync.drain()
        wait_clock.add_sem_waits(
            drain_inst.ins, ScopedClock({None: tick_clock.global_clock})
        )
        si = drain_inst.ins.sync_info
        if si is not None and si.on_wait and len(si.on_wait) > 1:
            extra = list(si.on_wait[1:])
            del si.on_wait[1:]
            for w in extra:
                d = nc.sync.drain()
                dsi = d.ins.sync_info
                if dsi is None:
                    d.ins.sync_info = mybir.SyncInfo(on_wait=[w], on_update=[])
                else:
                    dsi.on_wait.append(w)
        nc.all_engine_barrier()
        assert self.sems is not None
        popped = nc._tile_sem_poison_stack.pop()
        assert popped is self._sem_poison
        nc.clear_and_free_semaphores(list(self.sems.allocated().values()))
        nc.all_engine_barrier()

    tile.TileContext._drain_and_barrier = _drain_and_barrier_split
    tile.TileContext._drain_patched = True


def _split_multi_waits(nc):
    """Hoist extra sem-waits onto standalone event-sem instructions."""
    n_split = 0
    for f in nc.m.functions:
        for bb in f.blocks:
            new_list = []
            for inst in bb.instructions:
                si = inst.sync_info
                if si is not None and si.on_wait is not None and len(si.on_wait) > 1:
                    extra = list(si.on_wait[:-1])
                    keep = [si.on_wait[-1]]
                    for w in extra:
                        ev = mybir.InstEventSemaphore(
                            name=f"{inst.name}-wsplit{n_split}",
                            ins=[], outs=[],
                            sync_info=mybir.SyncInfo(on_wait=[w], on_update=[]),
                        )
                        ev.engine = inst.engine
                        nc.register_instruction(ev)
                        new_list.append(ev)
                        n_split += 1
                    del si.on_wait[:]
                    si.on_wait.extend(keep)
                new_list.append(inst)
            bb.instructions[:] = new_list
    return n_split


def _build_module():
    _patch_tile_drain()
    nc = bass.Bass()
    xin = nc.declare_dram_parameter("xin", [BPC, D, N], dt.float32, isOutput=False)
    cw = nc.declare_dram_parameter("cw", [K, D], dt.float32, isOutput=False)
    s_col = nc.declare_dram_parameter("s_col", [K, 1], dt.float32, isOutput=False)
    s_row = nc.declare_dram_parameter("s_row", [1, K], dt.float32, isOutput=False)
    eout = nc.declare_dram_parameter("eout", [BPC, K, D], dt.float32, isOutput=True)

    f16, f32 = dt.float16, dt.float32
    AX = mybir.AxisListType.X
    EXP = mybir.ActivationFunctionType.Exp

    with tile.TileContext(nc) as tc:
        with ExitStack() as ctx:
            singles = ctx.enter_context(tc.tile_pool(name="singles", bufs=1))
            psprep = ctx.enter_context(tc.tile_pool(name="psprep", bufs=1, space="PSUM"))

            # ---- one-time prep from codewords/scale ----
            cw_sb = singles.tile([K, D], f32)
            nc.sync.dma_start(cw_sb[:], cw[:])
            scol_sb = singles.tile([K, 1], f32)
            nc.sync.dma_start(scol_sb[:], s_col[:])
            srow_sb = singles.tile([1, K], f32)
            nc.sync.dma_start(srow_sb[:], s_row[:])

            # G16 [128, 2, K]: G[p, c, k] = -2 s_k c[k, c*128+p]
            w1 = singles.tile([K, D], f32)
            nc.vector.tensor_scalar_mul(w1[:], cw_sb[:], scol_sb[:])
            w2 = singles.tile([K, D], f32)
            nc.vector.tensor_scalar_mul(w2[:], w1[:], -2.0)
            w16 = singles.tile([K, D], f16)
            nc.vector.tensor_copy(w16[:], w2[:])
            g16 = singles.tile([128, 2 * K], f16)
            # g16[32j+w, c, k] = w16[k, c*128+32j+w] via DVE 32x32-block
            # transposes (keeps the prep transpose off the DMA fence chain)
            g3t = g16[:].rearrange("p (c k) -> p c k", k=K)
            wv = w16[:].rearrange("k (c j w) -> k c j w", j=4, w=32)
            for j in range(4):
                nc.vector.transpose(g3t[32 * j:32 * (j + 1), :, :], wv[:, :, j, :])

            # S16 [128, K]: every row = s_k (fp16)
            ones_row16 = singles.tile([1, 128], f16)
            nc.vector.memset(ones_row16[:], 1.0)
            srow16 = singles.tile([1, K], f16)
            nc.vector.tensor_copy(srow16[:], srow_sb[:])
            ps_s = psprep.tile([128, K], f32)
            nc.tensor.matmul(ps_s[:], ones_row16[:], srow16[:], start=True, stop=True,
                             skip_group_check=True)
            s16 = singles.tile([128, K], f16)
            nc.vector.tensor_copy(s16[:], ps_s[:])

            # bias [K, 1] = s_k * ||c_k||^2
            csq = singles.tile([K, D], f32)
            nc.vector.tensor_mul(csq[:], cw_sb[:], cw_sb[:])
            sqc = singles.tile([K, 1], f32)
            nc.vector.reduce_sum(
                sqc[:].rearrange("k (o p) -> k o p", o=1),
                csq[:].rearrange("k (o d) -> k o d", o=1), axis=AX)
            bias = singles.tile([K, 1], f32)
            nc.vector.tensor_mul(bias[:], sqc[:], scol_sb[:])

            ones_col16 = singles.tile([128, 1], f16)
            nc.vector.memset(ones_col16[:], 1.0)

            # ---- per-batch pools ----
            xpool = ctx.enter_context(tc.tile_pool(name="x16", bufs=2))
            xtpool = ctx.enter_context(tc.tile_pool(name="xt", bufs=2))
            sqpool = ctx.enter_context(tc.tile_pool(name="xsq", bufs=2))
            upool = ctx.enter_context(tc.tile_pool(name="u16", bufs=1))
            utpool = ctx.enter_context(tc.tile_pool(name="ut", bufs=2))
            apool = ctx.enter_context(tc.tile_pool(name="a16", bufs=2))
            npool = ctx.enter_context(tc.tile_pool(name="nrm", bufs=3))
            opool = ctx.enter_context(tc.tile_pool(name="out", bufs=1))
            psl = ctx.enter_context(tc.tile_pool(name="psl", bufs=3, space="PSUM"))
            pse = ctx.enter_context(tc.tile_pool(name="pse", bufs=2, space="PSUM"))

            efs = []
            for b in range(BPC):
                x0 = xpool.tile([128, N], f16, tag="x0")
                nc.gpsimd.dma_start(x0[:], xin[b, 0:128, :])
                x1 = xpool.tile([128, N], f16, tag="x1")
                nc.gpsimd.dma_start(x1[:], xin[b, 128:256, :])

                xt0 = xtpool.tile([128, NCHUNK * 128], f16, tag="xt0")
                xt0v = xt0[:].rearrange("p (t w) -> p t w", w=128)
                nc.sync.dma_start_transpose(xt0v, x0[:])
                xt1 = xtpool.tile([128, NCHUNK * 128], f16, tag="xt1")
                xt1v = xt1[:].rearrange("p (t w) -> p t w", w=128)
                nc.sync.dma_start_transpose(xt1v, x1[:])

                # U in two half tiles so the DVE transpose of half A can
                # start while exp tiles of half B are still being produced
                u16a = upool.tile([K, N // 2], f16, tag="ua")
                u16b = upool.tile([K, N // 2], f16, tag="ub")

                def u_slice(i):
                    lo = i * TN
                    if lo < N // 2:
                        return u16a[:, lo:lo + TN]
                    return u16b[:, lo - N // 2:lo - N // 2 + TN]

                for g in range(NT // SQG):
                    xsq = sqpool.tile([128, 2 * SQG * TN], f16, tag="xsq")
                    xsq3 = xsq[:].rearrange("p (c m) -> p c m", c=2)
                    sl = bass.ts(g, SQG * TN)
                    nc.vector.tensor_mul(xsq3[:, 0, :], x0[:, sl], x0[:, sl])
                    nc.vector.tensor_mul(xsq3[:, 1, :], x1[:, sl], x1[:, sl])
                    for j in range(SQG):
                        i = g * SQG + j
                        pl = psl.tile([K, TN], f32)
                        xs = bass.ts(i, TN)
                        js = bass.ts(j, TN)
                        nc.tensor.matmul(pl[:], g16[:].rearrange("p (c k) -> p c k", k=K)[:, 0, :],
                                         x0[:, xs], start=True, stop=False, skip_group_check=True)
                        nc.tensor.matmul(pl[:], g16[:].rearrange("p (c k) -> p c k", k=K)[:, 1, :],
                                         x1[:, xs], start=False, stop=False, skip_group_check=True)
                        nc.tensor.matmul(pl[:], s16[:], xsq3[:, 0, js],
                                         start=False, stop=False, skip_group_check=True)
                        nc.tensor.matmul(pl[:], s16[:], xsq3[:, 1, js],
                                         start=False, stop=True, skip_group_check=True)
                        nc.scalar.activation(u_slice(i), pl[:], EXP, bias=bias[:], scale=1.0)

                ut = utpool.tile([128, NCHUNK * K], f16)
                ut3 = ut[:].rearrange("p (t k) -> p t k", k=K)
                # ut3[32j+w, t, k] = U[k, t*128 + 32j + w]:
                # 32x32-block DVE stream-transposes, per j and per U-half
                HT = NCHUNK // 2
                for uh, ut16 in ((0, u16a), (1, u16b)):
                    uv = ut16[:].rearrange("k (t j w) -> k t j w", j=4, w=32)
                    for j in range(4):
                        nc.vector.transpose(
                            ut3[32 * j:32 * (j + 1), uh * HT:(uh + 1) * HT, :],
                            uv[:, :, j, :])

                # per-group A tiles so E-matmuls start after the first
                # normalize group instead of after all 72 chunks
                NG = 4
                pe = pse.tile([K, 257], f32)
                for g in range(NCHUNK // NG):
                    den = npool.tile([128, NG], f32, tag="den")
                    den3 = den[:].rearrange("p (t o) -> p t o", o=1)
                    sl3 = ut3[:, g * NG:(g + 1) * NG, :]
                    nc.vector.reduce_sum(den3, sl3, axis=AX)
                    rec = npool.tile([128, NG], f32, tag="rec")
                    nc.vector.reciprocal(rec[:], den[:])
                    recb = rec[:].rearrange("p (t o) -> p t o", o=1).broadcast_to((128, NG, K))
                    ag = apool.tile([128, NG * K], f16, tag=f"a{g}")
                    ag3 = ag[:].rearrange("p (t k) -> p t k", k=K)
                    nc.vector.tensor_mul(ag3[:], sl3, recb)
                    for tt in range(NG):
                        t = g * NG + tt
                        sp = (t == NCHUNK - 1)
                        # start=True clears has_written for the whole PSUM
                        # bank, so only the very first matmul touching this
                        # bank may set it.
                        nc.tensor.matmul(pe[:, 0:128], ag3[:, tt, :], xt0v[:, t, :],
                                         start=(t == 0), stop=sp, skip_group_check=True)
                        nc.tensor.matmul(pe[:, 128:256], ag3[:, tt, :], xt1v[:, t, :],
                                         start=False, stop=sp, skip_group_check=True)
                        nc.tensor.matmul(pe[:, 256:257], ag3[:, tt, :], ones_col16[:],
                                         start=False, stop=sp, skip_group_check=True)

                cs = opool.tile([K, D], f32, tag=f"cs{b}")
                nc.vector.tensor_scalar_mul(cs[:], cw_sb[:], pe[:, 256:257])
                ef = opool.tile([K, D], f32, tag=f"ef{b}")
                nc.vector.tensor_sub(ef[:], pe[:, 0:256], cs[:])
                efs.append(ef)

            # all output stores after the last transpose: each interleaved
            # DMA around the transpose fence costs a ~3us chain link
            for b, ef in enumerate(efs):
                nc.sync.dma_start(eout[b], ef[:])

    _split_multi_waits(nc)
    return nc


_NC_CACHE = None


def _run(X, codewords, scale, trace=False, tmpdir=None):
    global _NC_CACHE
    if _NC_CACHE is None:
        _NC_CACHE = _build_module()
    nc = _NC_CACHE
    Xr = np.ascontiguousarray(X.reshape(B, D, N), dtype=np.float32)
    cw = np.ascontiguousarray(codewords, dtype=np.float32)
    s = np.asarray(scale, dtype=np.float32).reshape(-1)
    in_maps = []
    for c in range(NCORES):
        in_maps.append({
            "xin": Xr[c * BPC:(c + 1) * BPC],
            "cw": cw,
            "s_col": np.ascontiguousarray(s.reshape(K, 1)),
            "s_row": np.ascontiguousarray(s.reshape(1, K)),
        })
    kr = run_bass_kernel_spmd(nc, in_maps, list(range(NCORES)),
                              trace=trace, tmpdir=tmpdir)
    out = np.concatenate([r["eout"] for r in kr.results], axis=0)
    return out.astype(np.float32), kr


def kernel(X, codewords, scale):
    out, _ = _run(X, codewords, scale)
    return out
